# revision 8
# baseline (speedup 1.0000x reference)
"""Trainium2 Bass kernel: fused attention block (QKV proj -> MHA -> out proj).

Reference (per batch item b, NUM_HEADS=12, Dh=64):
    qkv = x @ W_qkv; q,k,v per head
    attn = softmax(q @ k^T / 8) @ v
    out  = concat_heads(attn) @ W_proj + b_proj

Sharding: data-parallel over batch across 8 NeuronCores (128 batch items
per core), weights replicated. One SPMD Bass program, per-core inputs.

Per-core plan (128 batches, groups of G=8 batches = 392 tokens):
  A. DMA x token-major, PE-transpose to feature-major xT
  B. q,k GEMM feature-major: psum[128co, T] = Wqkv_tile.T @ xT   (float32r)
  C. v GEMM token-major: psum[tok, 384] = xT_toktile.T @ Wv; distribute
     per-batch [49, 12, 65] ([v | ones]) blocks via SBUF->SBUF DMA
  D. attention per (head, batch), matmuls padded to even moving dim
     (float32r requires even N):
       sT  = k_slice.T @ q_slice        [49m, 50]  (scores transposed)
       eT  = exp(sT/8)                  ACT, f32r out
       po  = [v|ones].T @ eT            [65, 50]: unnorm out^T + row-sums
       rr  = 1/po[64]                   DVE reciprocal
       pbc = ones1.T @ rr               K=1 broadcast matmul [128, T]
       unT = po[0:64] * pbc             normalized attn^T, feature-major
  E. proj GEMM token-major: psum[tok, 384] = unT_toktile.T @ Wproj + bias
     -> DMA out (contiguous rows)
"""
import sys

sys.path.insert(0, "/opt/trn_rl_repo")

import numpy as np

NUM_CORES = 8
B_CORE = 128          # batch items per core
SEQ = 49              # tokens per batch item
C = 768               # channels
H = 12                # heads
G = 8                 # batch items per group
T = SEQ * G           # 392 tokens per group (even)
TP = T + 2            # padded q/k tile width
TOK = B_CORE * SEQ    # 6272 tokens per core
N_GROUPS = B_CORE // G

_CACHE = {}


def _build():
    import concourse.bacc as bacc
    import concourse.mybir as mybir
    import concourse.tile as tile

    F32 = mybir.dt.float32
    F32R = mybir.dt.float32r
    EXP = mybir.ActivationFunctionType.Exp

    nc = bacc.Bacc("TRN2", target_bir_lowering=False)

    d_x = nc.declare_dram_parameter("x", [TOK, C], F32, isOutput=False)
    d_wqkv = nc.declare_dram_parameter("wqkv", [C, 3 * C], F32R, isOutput=False)
    d_wproj = nc.declare_dram_parameter("wproj", [C, C], F32R, isOutput=False)
    d_bias = nc.declare_dram_parameter("bias", [1, C], F32R, isOutput=False)
    d_ones1 = nc.declare_dram_parameter("ones1", [1, 128], F32R, isOutput=False)
    d_ident = nc.declare_dram_parameter("ident", [128, 128], F32, isOutput=False)
    d_vones = nc.declare_dram_parameter("vones", [SEQ, G * H * 65], F32R,
                                        isOutput=False)
    d_out = nc.declare_dram_parameter("out", [TOK, C], F32, isOutput=True)

    # token tiles within a group
    tts = []
    o = 0
    while o < T:
        tts.append((o, min(128, T - o)))
        o += 128

    with tile.TileContext(nc) as tc, \
         nc.allow_low_precision(reason="float32r storage for full-rate matmul"):
        with tc.tile_pool(name="wres", bufs=1) as wres, \
             tc.tile_pool(name="xtm", bufs=4) as p_xtm, \
             tc.tile_pool(name="xT", bufs=1) as p_xT, \
             tc.tile_pool(name="qk", bufs=1) as p_qk, \
             tc.tile_pool(name="vtok", bufs=4) as p_vtok, \
             tc.tile_pool(name="vsb", bufs=1) as p_vsb, \
             tc.tile_pool(name="eT", bufs=6) as p_eT, \
             tc.tile_pool(name="rr", bufs=2) as p_rr, \
             tc.tile_pool(name="bc", bufs=2) as p_bc, \
             tc.tile_pool(name="unT", bufs=1) as p_unT, \
             tc.tile_pool(name="osb", bufs=2) as p_osb, \
             tc.tile_pool(name="psA", bufs=2, space="PSUM") as psA, \
             tc.tile_pool(name="psB", bufs=2, space="PSUM") as psB, \
             tc.tile_pool(name="psS", bufs=2, space="PSUM") as psS, \
             tc.tile_pool(name="psO", bufs=2, space="PSUM") as psO:

            # ---- resident weights / constants ----
            w_qkv = []
            for ci in range(6):
                t = wres.tile([128, 3 * C], F32R, tag=f"wqkv{ci}")
                nc.sync.dma_start(t[:], d_wqkv[128 * ci:128 * (ci + 1), :])
                w_qkv.append(t)
            w_proj = []
            for ci in range(6):
                t = wres.tile([128, C], F32R, tag=f"wproj{ci}")
                nc.sync.dma_start(t[:], d_wproj[128 * ci:128 * (ci + 1), :])
                w_proj.append(t)
            ones1 = wres.tile([1, 128], F32R, tag="ones1")
            nc.sync.dma_start(ones1[:], d_ones1[:])
            ident = wres.tile([128, 128], F32, tag="ident")
            nc.sync.dma_start(ident[:], d_ident[:])
            bias_sb = wres.tile([1, C], F32R, tag="bias_sb")
            nc.sync.dma_start(bias_sb[:], d_bias[:])
            bias_bc = wres.tile([128, C], F32, tag="bias_bc")
            for half in range(2):
                pb = psB.tile([128, 384], F32, tag="psB")
                nc.tensor.matmul(pb[:], ones1[:],
                                 bias_sb[:, 384 * half:384 * (half + 1)],
                                 start=True, stop=True)
                nc.scalar.copy(bias_bc[:, 384 * half:384 * (half + 1)], pb[:])

            for g in range(N_GROUPS):
                r0 = g * T  # first token row of group

                # ---- A: load x token-major, transpose to xT ----
                x_tm = []
                for (to, tk) in tts:
                    t = p_xtm.tile([128, C], F32, tag="xtm")
                    nc.sync.dma_start(t[:tk, :], d_x[r0 + to:r0 + to + tk, :])
                    x_tm.append(t)
                xT = [p_xT.tile([128, T], F32R, tag=f"xT{ci}", name=f"xT{ci}")
                      for ci in range(6)]
                for tti, (to, tk) in enumerate(tts):
                    for ci in range(6):
                        pt = psB.tile([128, 384], F32, tag="psB")
                        nc.tensor.transpose(
                            pt[:, :tk],
                            x_tm[tti][:tk, 128 * ci:128 * (ci + 1)],
                            ident[:tk, :tk])
                        nc.vector.tensor_copy(xT[ci][:, to:to + tk], pt[:, :tk])

                # ---- B: q,k GEMM feature-major ----
                qk = []
                for j in range(12):
                    pq = psA.tile([128, TP], F32, tag="psA")
                    for ci in range(6):
                        nc.tensor.matmul(
                            pq[:, :T],
                            w_qkv[ci][:, 128 * j:128 * (j + 1)],
                            xT[ci][:, :T],
                            start=(ci == 0), stop=(ci == 5))
                    t = p_qk.tile([128, TP], F32R, tag=f"qk{j}")
                    nc.vector.tensor_copy(t[:, :T], pq[:, :T])
                    nc.vector.tensor_copy(t[:, T:T + 2], pq[:, :2])  # finite pad
                    qk.append(t)

                # ---- C: v GEMM token-major + per-batch distribute ----
                v_tok = []
                for tti, (to, tk) in enumerate(tts):
                    t = p_vtok.tile([128, C], F32R, tag="vtok")
                    for half in range(2):
                        pv = psB.tile([128, 384], F32, tag="psB")
                        for ci in range(6):
                            nc.tensor.matmul(
                                pv[:tk, :],
                                xT[ci][:, to:to + tk],
                                w_qkv[ci][:, 1536 + 384 * half:
                                           1536 + 384 * (half + 1)],
                                start=(ci == 0), stop=(ci == 5))
                        nc.vector.tensor_copy(
                            t[:tk, 384 * half:384 * (half + 1)], pv[:tk, :])
                    v_tok.append(t)
                v_sb = p_vsb.tile([SEQ, G * H * 65], F32R, tag="vsb")
                nc.sync.dma_start(v_sb[:], d_vones[:])  # [v|ones] ones pattern
                v4 = v_sb.rearrange("p (g h c) -> p g h c", h=H, c=65)
                for b in range(G):
                    t0 = b * SEQ
                    done = 0
                    while done < SEQ:
                        tti = (t0 + done) // 128
                        to, tk = tts[tti]
                        src_r0 = t0 + done - to
                        n = min(SEQ - done, tk - src_r0)
                        nc.sync.dma_start(
                            v4[done:done + n, b, :, 0:64],
                            v_tok[tti][src_r0:src_r0 + n, :]
                            .rearrange("p (h c) -> p h c", c=64))
                        done += n

                # ---- D: attention cells ----
                unT = [p_unT.tile([128, T], F32R, tag=f"unT{ci}", name=f"unT{ci}")
                       for ci in range(6)]
                for j in range(6):
                    for par in range(2):
                        h = 2 * j + par
                        pl = 64 * par
                        # 50-wide per-batch slots: f32r matmul psum output
                        # offsets must be even
                        po = psO.tile([65, 50 * G], F32, tag="psO")
                        for b in range(G):
                            ps = psS.tile([SEQ, 50], F32, tag="psS")
                            nc.tensor.matmul(
                                ps[:],
                                qk[6 + j][pl:pl + 64, 49 * b:49 * b + 49],
                                qk[j][pl:pl + 64, 49 * b:49 * b + 50],
                                start=True, stop=True)
                            eT = p_eT.tile([SEQ, 50], F32R, tag="eT")
                            nc.scalar.activation(eT[:], ps[:], EXP, scale=0.125)
                            nc.tensor.matmul(
                                po[:, 50 * b:50 * b + 50],
                                v4[0:SEQ, b, h, 0:65],
                                eT[:],
                                start=True, stop=True)
                        po_r = po[64:65, :].rearrange(
                            "p (b c) -> p b c", c=50)[:, :, 0:49]
                        po_d = po[0:64, :].rearrange(
                            "p (b c) -> p b c", c=50)[:, :, 0:49]
                        rr = p_rr.tile([1, T], F32R, tag="rr")
                        nc.vector.reciprocal(
                            rr.rearrange("p (b c) -> p b c", c=49), po_r)
                        pbc = psA.tile([128, TP], F32, tag="psA")
                        nc.tensor.matmul(pbc[:, :T], ones1[:], rr[:, :T],
                                         start=True, stop=True)
                        bc = p_bc.tile([128, T], F32, tag="bc")
                        nc.scalar.copy(bc[:, :T], pbc[:, :T])
                        nc.vector.tensor_mul(
                            out=unT[j][pl:pl + 64, 0:T].rearrange(
                                "p (b c) -> p b c", c=49),
                            in0=po_d,
                            in1=bc[pl:pl + 64, :T].rearrange(
                                "p (b c) -> p b c", c=49))

                # ---- E: proj GEMM + bias -> out ----
                for tti, (to, tk) in enumerate(tts):
                    osb = p_osb.tile([128, C], F32, tag="osb")
                    for half in range(2):
                        pp = psB.tile([128, 384], F32, tag="psB")
                        for ci in range(6):
                            nc.tensor.matmul(
                                pp[:tk, :],
                                unT[ci][:, to:to + tk],
                                w_proj[ci][:, 384 * half:384 * (half + 1)],
                                start=(ci == 0), stop=(ci == 5))
                        nc.vector.tensor_add(
                            out=osb[:tk, 384 * half:384 * (half + 1)],
                            in0=pp[:tk, :],
                            in1=bias_bc[:tk, 384 * half:384 * (half + 1)])
                    nc.sync.dma_start(d_out[r0 + to:r0 + to + tk, :],
                                      osb[:tk, :])

    nc.compile()
    return nc


def kernel(x, W_qkv, W_proj, b_proj):
    from concourse.bass_utils import run_bass_kernel_spmd

    if "nc" not in _CACHE:
        _CACHE["nc"] = _build()
    nc = _CACHE["nc"]

    x = np.ascontiguousarray(np.asarray(x, dtype=np.float32))
    B, N, Cc = x.shape
    ones1 = np.ones((1, 128), dtype=np.float32)
    ident = np.eye(128, dtype=np.float32)
    vones = np.zeros((SEQ, G * H * 65), dtype=np.float32)
    vones.reshape(SEQ, G * H, 65)[:, :, 64] = 1.0
    W_qkv = np.ascontiguousarray(np.asarray(W_qkv, dtype=np.float32))
    W_proj = np.ascontiguousarray(np.asarray(W_proj, dtype=np.float32))
    bias = np.ascontiguousarray(
        np.asarray(b_proj, dtype=np.float32).reshape(1, Cc))

    in_maps = []
    for i in range(NUM_CORES):
        in_maps.append({
            "x": np.ascontiguousarray(
                x[i * B_CORE:(i + 1) * B_CORE].reshape(TOK, Cc)),
            "wqkv": W_qkv, "wproj": W_proj, "bias": bias,
            "ones1": ones1, "ident": ident, "vones": vones,
        })
    res = run_bass_kernel_spmd(nc, in_maps, list(range(NUM_CORES)))
    out = np.empty((B, N, Cc), dtype=np.float32)
    for i in range(NUM_CORES):
        out[i * B_CORE:(i + 1) * B_CORE] = res.results[i]["out"].reshape(
            B_CORE, N, Cc)
    return out


# revision 13
# speedup vs baseline: 1.2993x; 1.2993x over previous
"""Trainium2 Bass kernel: fused attention block (QKV proj -> MHA -> out proj).

Reference (per batch item b, NUM_HEADS=12, Dh=64):
    qkv = x @ W_qkv; q,k,v per head
    attn = softmax(q @ k^T / 8) @ v
    out  = concat_heads(attn) @ W_proj + b_proj

Sharding: data-parallel over batch across 8 NeuronCores (128 batch items
per core), weights replicated. One SPMD Bass program, per-core inputs.

Per-core plan (128 batches, groups of G=8 batches = 392 tokens):
  A. DMA x token-major, PE-transpose to feature-major xT
  B. q,k GEMM feature-major: psum[128co, T] = Wqkv_tile.T @ xT   (float32r)
  C. v GEMM token-major: psum[tok, 384] = xT_toktile.T @ Wv; distribute
     per-batch [49, 12, 65] ([v | ones]) blocks via SBUF->SBUF DMA
  D. attention per (head, batch), matmuls padded to even moving dim
     (float32r requires even N):
       sT  = k_slice.T @ q_slice        [49m, 50]  (scores transposed)
       eT  = exp(sT/8)                  ACT, f32r out
       po  = [v|ones].T @ eT            [65, 50]: unnorm out^T + row-sums
       rr  = 1/po[64]                   DVE reciprocal
       pbc = ones1.T @ rr               K=1 broadcast matmul [128, T]
       unT = po[0:64] * pbc             normalized attn^T, feature-major
  E. proj GEMM token-major: psum[tok, 384] = unT_toktile.T @ Wproj + bias
     -> DMA out (contiguous rows)
"""
import sys

sys.path.insert(0, "/opt/trn_rl_repo")

import numpy as np

NUM_CORES = 8
B_CORE = 128          # batch items per core
SEQ = 49              # tokens per batch item
C = 768               # channels
H = 12                # heads
G = 8                 # batch items per group
T = SEQ * G           # 392 tokens per group (even)
TP = T + 2            # padded q/k tile width
TOK = B_CORE * SEQ    # 6272 tokens per core
N_GROUPS = B_CORE // G

_CACHE = {}


def _build():
    import concourse.bacc as bacc
    import concourse.mybir as mybir
    import concourse.tile as tile

    F32 = mybir.dt.float32
    F32R = mybir.dt.float32r
    EXP = mybir.ActivationFunctionType.Exp

    nc = bacc.Bacc("TRN2", target_bir_lowering=False)

    d_x = nc.declare_dram_parameter("x", [TOK, C], F32, isOutput=False)
    d_wqkv = nc.declare_dram_parameter("wqkv", [C, 3 * C], F32R, isOutput=False)
    d_wproj = nc.declare_dram_parameter("wproj", [C, C], F32R, isOutput=False)
    d_bias = nc.declare_dram_parameter("bias", [1, C], F32R, isOutput=False)
    d_ones1 = nc.declare_dram_parameter("ones1", [1, 128], F32R, isOutput=False)
    d_ident = nc.declare_dram_parameter("ident", [128, 128], F32, isOutput=False)
    d_vones = nc.declare_dram_parameter("vones", [SEQ, G * H * 65], F32R,
                                        isOutput=False)
    d_sel2 = nc.declare_dram_parameter("sel2", [H, 6 * 128], F32R,
                                       isOutput=False)
    d_out = nc.declare_dram_parameter("out", [TOK, C], F32, isOutput=True)

    # token tiles within a group
    tts = []
    o = 0
    while o < T:
        tts.append((o, min(128, T - o)))
        o += 128

    with tile.TileContext(nc) as tc, \
         nc.allow_low_precision(reason="float32r storage for full-rate matmul"):
        with tc.tile_pool(name="wres", bufs=1) as wres, \
             tc.tile_pool(name="xtm", bufs=4) as p_xtm, \
             tc.tile_pool(name="xT", bufs=1) as p_xT, \
             tc.tile_pool(name="qk", bufs=1) as p_qk, \
             tc.tile_pool(name="vtok", bufs=4) as p_vtok, \
             tc.tile_pool(name="vsb", bufs=1) as p_vsb, \
             tc.tile_pool(name="eT", bufs=6) as p_eT, \
             tc.tile_pool(name="rr", bufs=2) as p_rr, \
             tc.tile_pool(name="bc", bufs=2) as p_bc, \
             tc.tile_pool(name="unT", bufs=1) as p_unT, \
             tc.tile_pool(name="osb", bufs=2) as p_osb, \
             tc.tile_pool(name="psA", bufs=2, space="PSUM") as psA, \
             tc.tile_pool(name="psB", bufs=2, space="PSUM") as psB, \
             tc.tile_pool(name="psS", bufs=2, space="PSUM") as psS, \
             tc.tile_pool(name="psO", bufs=2, space="PSUM") as psO:

            # ---- resident weights / constants ----
            w_qkv = []
            for ci in range(6):
                t = wres.tile([128, 3 * C], F32R, tag=f"wqkv{ci}")
                nc.sync.dma_start(t[:], d_wqkv[128 * ci:128 * (ci + 1), :])
                w_qkv.append(t)
            w_proj = []
            for ci in range(6):
                t = wres.tile([128, C], F32R, tag=f"wproj{ci}")
                nc.sync.dma_start(t[:], d_wproj[128 * ci:128 * (ci + 1), :])
                w_proj.append(t)
            ones1 = wres.tile([1, 128], F32R, tag="ones1")
            nc.sync.dma_start(ones1[:], d_ones1[:])
            sel2 = wres.tile([H, 6 * 128], F32R, tag="sel2")
            nc.sync.dma_start(sel2[:], d_sel2[:])
            ident = wres.tile([128, 128], F32, tag="ident")
            nc.sync.dma_start(ident[:], d_ident[:])
            bias_sb = wres.tile([1, C], F32R, tag="bias_sb")
            nc.sync.dma_start(bias_sb[:], d_bias[:])
            bias_bc = wres.tile([128, C], F32, tag="bias_bc")
            for half in range(2):
                pb = psB.tile([128, 384], F32, tag="psB")
                nc.tensor.matmul(pb[:], ones1[:],
                                 bias_sb[:, 384 * half:384 * (half + 1)],
                                 start=True, stop=True)
                nc.scalar.copy(bias_bc[:, 384 * half:384 * (half + 1)], pb[:])

            for g in range(N_GROUPS):
                r0 = g * T  # first token row of group

                # ---- A: load x token-major, transpose to xT ----
                x_tm = []
                for (to, tk) in tts:
                    t = p_xtm.tile([128, C], F32, tag="xtm")
                    nc.sync.dma_start(t[:tk, :], d_x[r0 + to:r0 + to + tk, :])
                    x_tm.append(t)
                xT = [p_xT.tile([128, T], F32R, tag=f"xT{ci}", name=f"xT{ci}")
                      for ci in range(6)]
                for tti, (to, tk) in enumerate(tts):
                    for ci in range(6):
                        pt = psB.tile([128, 384], F32, tag="psB")
                        nc.tensor.transpose(
                            pt[:, :tk],
                            x_tm[tti][:tk, 128 * ci:128 * (ci + 1)],
                            ident[:tk, :tk])
                        nc.vector.tensor_copy(xT[ci][:, to:to + tk], pt[:, :tk])

                # ---- B: q,k GEMM feature-major ----
                qk = []
                for j in range(12):
                    pq = psA.tile([128, TP], F32, tag="psA")
                    for ci in range(6):
                        nc.tensor.matmul(
                            pq[:, :T],
                            w_qkv[ci][:, 128 * j:128 * (j + 1)],
                            xT[ci][:, :T],
                            start=(ci == 0), stop=(ci == 5))
                    t = p_qk.tile([128, TP], F32R, tag=f"qk{j}")
                    nc.vector.tensor_copy(t[:, :T], pq[:, :T])
                    nc.vector.tensor_copy(t[:, T:T + 2], pq[:, :2])  # finite pad
                    qk.append(t)

                # ---- C: v GEMM token-major + per-batch distribute ----
                v_tok = []
                for tti, (to, tk) in enumerate(tts):
                    t = p_vtok.tile([128, C], F32R, tag="vtok")
                    for half in range(2):
                        pv = psB.tile([128, 384], F32, tag="psB")
                        for ci in range(6):
                            nc.tensor.matmul(
                                pv[:tk, :],
                                xT[ci][:, to:to + tk],
                                w_qkv[ci][:, 1536 + 384 * half:
                                           1536 + 384 * (half + 1)],
                                start=(ci == 0), stop=(ci == 5))
                        nc.vector.tensor_copy(
                            t[:tk, 384 * half:384 * (half + 1)], pv[:tk, :])
                    v_tok.append(t)
                v_sb = p_vsb.tile([SEQ, G * H * 65], F32R, tag="vsb")
                nc.sync.dma_start(v_sb[:], d_vones[:])  # [v|ones] ones pattern
                v4 = v_sb.rearrange("p (g h c) -> p g h c", h=H, c=65)
                for b in range(G):
                    t0 = b * SEQ
                    done = 0
                    while done < SEQ:
                        tti = (t0 + done) // 128
                        to, tk = tts[tti]
                        src_r0 = t0 + done - to
                        n = min(SEQ - done, tk - src_r0)
                        nc.sync.dma_start(
                            v4[done:done + n, b, :, 0:64],
                            v_tok[tti][src_r0:src_r0 + n, :]
                            .rearrange("p (h c) -> p h c", c=64))
                        done += n

                # ---- D: attention cells ----
                unT = [p_unT.tile([128, T], F32R, tag=f"unT{ci}", name=f"unT{ci}")
                       for ci in range(6)]
                rgrp = p_rr.tile([H, T], F32, tag="rgrp")
                for j in range(6):
                    for par in range(2):
                        h = 2 * j + par
                        pl = 64 * par
                        # 50-wide per-batch slots: f32r matmul psum output
                        # offsets must be even
                        ps = psS.tile([SEQ, 50 * G], F32, tag="psS")
                        for b in range(G):
                            nc.tensor.matmul(
                                ps[:, 50 * b:50 * b + 50],
                                qk[6 + j][pl:pl + 64, 49 * b:49 * b + 49],
                                qk[j][pl:pl + 64, 49 * b:49 * b + 50],
                                start=True, stop=True)
                        eT = p_eT.tile([SEQ, 50 * G], F32R, tag="eT")
                        nc.scalar.activation(eT[:], ps[:], EXP, scale=0.125)
                        po = psO.tile([65, 50 * G], F32, tag="psO")
                        for b in range(G):
                            nc.tensor.matmul(
                                po[:, 50 * b:50 * b + 50],
                                v4[0:SEQ, b, h, 0:65],
                                eT[:, 50 * b:50 * b + 50],
                                start=True, stop=True)
                        # unnormalized out^T -> unT (contiguous columns);
                        # denominator row -> r1 -> rgrp row h (DMA shift)
                        nc.vector.tensor_copy(
                            unT[j][pl:pl + 64, 0:T].rearrange(
                                "p (b c) -> p b c", c=49),
                            po[0:64, :].rearrange(
                                "p (b c) -> p b c", c=50)[:, :, 0:49])
                        r1 = p_bc.tile([1, T], F32, tag="r1")
                        nc.scalar.copy(
                            r1.rearrange("p (b c) -> p b c", c=49),
                            po[64:65, :].rearrange(
                                "p (b c) -> p b c", c=50)[:, :, 0:49])
                        nc.sync.dma_start(rgrp[h:h + 1, :], r1[:])
                # one batched reciprocal for all 12 heads of the group
                rr = p_rr.tile([H, T], F32R, tag="rr")
                nc.vector.reciprocal(rr[:], rgrp[:])
                for j in range(6):
                    pbc = psA.tile([128, TP], F32, tag="psA")
                    nc.tensor.matmul(pbc[:, :T], sel2[:, 128 * j:128 * (j + 1)],
                                     rr[:], start=True, stop=True)
                    bc = p_bc.tile([128, T], F32, tag="bc")
                    nc.scalar.copy(bc[:], pbc[:, :T])
                    nc.vector.tensor_mul(out=unT[j][:, :], in0=unT[j][:, :],
                                         in1=bc[:])

                # ---- E: proj GEMM + bias -> out ----
                for tti, (to, tk) in enumerate(tts):
                    osb = p_osb.tile([128, C], F32, tag="osb")
                    for half in range(2):
                        pp = psB.tile([128, 384], F32, tag="psB")
                        for ci in range(6):
                            nc.tensor.matmul(
                                pp[:tk, :],
                                unT[ci][:, to:to + tk],
                                w_proj[ci][:, 384 * half:384 * (half + 1)],
                                start=(ci == 0), stop=(ci == 5))
                        nc.vector.tensor_add(
                            out=osb[:tk, 384 * half:384 * (half + 1)],
                            in0=pp[:tk, :],
                            in1=bias_bc[:tk, 384 * half:384 * (half + 1)])
                    nc.sync.dma_start(d_out[r0 + to:r0 + to + tk, :],
                                      osb[:tk, :])

    nc.compile()
    return nc


def kernel(x, W_qkv, W_proj, b_proj):
    from concourse.bass_utils import run_bass_kernel_spmd

    if "nc" not in _CACHE:
        _CACHE["nc"] = _build()
    nc = _CACHE["nc"]

    x = np.ascontiguousarray(np.asarray(x, dtype=np.float32))
    B, N, Cc = x.shape
    ones1 = np.ones((1, 128), dtype=np.float32)
    ident = np.eye(128, dtype=np.float32)
    vones = np.zeros((SEQ, G * H * 65), dtype=np.float32)
    vones.reshape(SEQ, G * H, 65)[:, :, 64] = 1.0
    sel2 = np.zeros((H, 6 * 128), dtype=np.float32)
    for j in range(6):
        sel2[2 * j, 128 * j:128 * j + 64] = 1.0
        sel2[2 * j + 1, 128 * j + 64:128 * (j + 1)] = 1.0
    W_qkv = np.ascontiguousarray(np.asarray(W_qkv, dtype=np.float32))
    W_proj = np.ascontiguousarray(np.asarray(W_proj, dtype=np.float32))
    bias = np.ascontiguousarray(
        np.asarray(b_proj, dtype=np.float32).reshape(1, Cc))

    in_maps = []
    for i in range(NUM_CORES):
        in_maps.append({
            "x": np.ascontiguousarray(
                x[i * B_CORE:(i + 1) * B_CORE].reshape(TOK, Cc)),
            "wqkv": W_qkv, "wproj": W_proj, "bias": bias,
            "ones1": ones1, "ident": ident, "vones": vones, "sel2": sel2,
        })
    res = run_bass_kernel_spmd(nc, in_maps, list(range(NUM_CORES)))
    out = np.empty((B, N, Cc), dtype=np.float32)
    for i in range(NUM_CORES):
        out[i * B_CORE:(i + 1) * B_CORE] = res.results[i]["out"].reshape(
            B_CORE, N, Cc)
    return out


# revision 15
# speedup vs baseline: 1.4571x; 1.1214x over previous
"""Trainium2 Bass kernel: fused attention block (QKV proj -> MHA -> out proj).

Reference (per batch item b, NUM_HEADS=12, Dh=64):
    qkv = x @ W_qkv; q,k,v per head
    attn = softmax(q @ k^T / 8) @ v
    out  = concat_heads(attn) @ W_proj + b_proj

Sharding: data-parallel over batch across 8 NeuronCores (128 batch items
per core), weights replicated. One SPMD Bass program, per-core inputs.

Per-core plan (128 batches, groups of G=8 batches = 392 tokens).
All matmuls use float32r (full PE rate, ~1.6e-4 rel err; requires even
moving dim N and even psum column offsets -> 50-wide per-batch slots).

  A. DMA x token-major, PE-transpose to feature-major xT
  B. q,k GEMM feature-major: psum[128co, T] = Wqkv_tile.T @ xT.
     q co-tiles stored naturally [128, T+2]; k co-tiles scattered into
     block-diagonal form kbd[j]: per batch a [128, 98] block with
     k_h(2j) in rows 0:64 cols 0:49 and k_h(2j+1) in rows 64:128
     cols 49:98 (zeros elsewhere, pre-loaded once from a host constant).
  C. v GEMM token-major -> scratch; SBUF->SBUF DMA scatter into vbd[j]:
     per batch a [98, 128] block with v_h(2j) rows 0:49 cols 0:64 and
     v_h(2j+1) rows 49:98 cols 64:128 (zeros preloaded once).
  D. attention per (head-pair j, batch): both heads in one matmul chain:
       sT2 = kbd_b.T @ q_pair          [98, 50]  scores, heads stacked
       eT2 = exp(sT2/8)                ACT, one op per pair-cell [98,400]
       r2  = onesbd.T @ eT2            [2, 400]  row sums per head
       po  = vbd_b.T @ eT2             [128, 50] unnorm out^T, both heads
     r rows gathered (ACT copy + SBUF-shift DMA) into rgrp[12, T];
     one batched reciprocal per group; per j: broadcast matmul
     (sel2 selector) -> bc[128, T]; unT[j] *= bc (in-place DVE).
  E. proj GEMM token-major: psum[tok, 384] = unT_toktile.T @ Wproj + bias
     -> DMA out (contiguous rows)
"""
import sys

sys.path.insert(0, "/opt/trn_rl_repo")

import numpy as np

NUM_CORES = 8
B_CORE = 128          # batch items per core
SEQ = 49              # tokens per batch item
C = 768               # channels
H = 12                # heads
G = 8                 # batch items per group
T = SEQ * G           # 392 tokens per group (even)
TP = T + 2            # padded q tile width
TOK = B_CORE * SEQ    # 6272 tokens per core
N_GROUPS = B_CORE // G
KBD_W = G * 98        # kbd block row width per j
VBD_W = G * 128       # vbd block row width per j

_CACHE = {}


def _consts():
    ones1 = np.ones((1, 128), dtype=np.float32)
    ident = np.eye(128, dtype=np.float32)
    sel2 = np.zeros((H, 6 * 128), dtype=np.float32)
    for j in range(6):
        sel2[2 * j, 128 * j:128 * j + 64] = 1.0
        sel2[2 * j + 1, 128 * j + 64:128 * (j + 1)] = 1.0
    onesbd = np.zeros((98, 2), dtype=np.float32)
    onesbd[0:49, 0] = 1.0
    onesbd[49:98, 1] = 1.0
    kbdz = np.zeros((128, 6 * KBD_W), dtype=np.float32)
    vbdz = np.zeros((98, 6 * VBD_W), dtype=np.float32)
    return {"ones1": ones1, "ident": ident, "sel2": sel2,
            "onesbd": onesbd, "kbdz": kbdz, "vbdz": vbdz}


def _build():
    import concourse.bacc as bacc
    import concourse.mybir as mybir
    import concourse.tile as tile

    F32 = mybir.dt.float32
    F32R = mybir.dt.float32r
    EXP = mybir.ActivationFunctionType.Exp

    nc = bacc.Bacc("TRN2", target_bir_lowering=False)

    d_x = nc.declare_dram_parameter("x", [TOK, C], F32, isOutput=False)
    d_wqkv = nc.declare_dram_parameter("wqkv", [C, 3 * C], F32R, isOutput=False)
    d_wproj = nc.declare_dram_parameter("wproj", [C, C], F32R, isOutput=False)
    d_bias = nc.declare_dram_parameter("bias", [1, C], F32R, isOutput=False)
    d_ones1 = nc.declare_dram_parameter("ones1", [1, 128], F32R, isOutput=False)
    d_ident = nc.declare_dram_parameter("ident", [128, 128], F32, isOutput=False)
    d_sel2 = nc.declare_dram_parameter("sel2", [H, 6 * 128], F32R, isOutput=False)
    d_onesbd = nc.declare_dram_parameter("onesbd", [98, 2], F32R, isOutput=False)
    d_kbdz = nc.declare_dram_parameter("kbdz", [128, 6 * KBD_W], F32R,
                                       isOutput=False)
    d_vbdz = nc.declare_dram_parameter("vbdz", [98, 6 * VBD_W], F32R,
                                       isOutput=False)
    d_out = nc.declare_dram_parameter("out", [TOK, C], F32, isOutput=True)

    # token tiles within a group
    tts = []
    o = 0
    while o < T:
        tts.append((o, min(128, T - o)))
        o += 128

    with tile.TileContext(nc) as tc, \
         nc.allow_low_precision(reason="float32r storage for full-rate matmul"):
        with tc.tile_pool(name="wres", bufs=1) as wres, \
             tc.tile_pool(name="xtm", bufs=4) as p_xtm, \
             tc.tile_pool(name="xT", bufs=1) as p_xT, \
             tc.tile_pool(name="qk", bufs=1) as p_qk, \
             tc.tile_pool(name="vscr", bufs=2) as p_vscr, \
             tc.tile_pool(name="eT", bufs=3) as p_eT, \
             tc.tile_pool(name="rr", bufs=1) as p_rr, \
             tc.tile_pool(name="bc", bufs=2) as p_bc, \
             tc.tile_pool(name="unT", bufs=1) as p_unT, \
             tc.tile_pool(name="osb", bufs=2) as p_osb, \
             tc.tile_pool(name="psA", bufs=2, space="PSUM") as psA, \
             tc.tile_pool(name="psB", bufs=2, space="PSUM") as psB, \
             tc.tile_pool(name="psS", bufs=2, space="PSUM") as psS, \
             tc.tile_pool(name="psO", bufs=2, space="PSUM") as psO:

            # ---- resident weights / constants ----
            w_qkv = []
            for ci in range(6):
                t = wres.tile([128, 3 * C], F32R, tag=f"wqkv{ci}")
                nc.sync.dma_start(t[:], d_wqkv[128 * ci:128 * (ci + 1), :])
                w_qkv.append(t)
            w_proj = []
            for ci in range(6):
                t = wres.tile([128, C], F32R, tag=f"wproj{ci}")
                nc.sync.dma_start(t[:], d_wproj[128 * ci:128 * (ci + 1), :])
                w_proj.append(t)
            ones1 = wres.tile([1, 128], F32R, tag="ones1")
            nc.sync.dma_start(ones1[:], d_ones1[:])
            sel2 = wres.tile([H, 6 * 128], F32R, tag="sel2")
            nc.sync.dma_start(sel2[:], d_sel2[:])
            onesbd = wres.tile([98, 2], F32R, tag="onesbd")
            nc.sync.dma_start(onesbd[:], d_onesbd[:])
            ident = wres.tile([128, 128], F32, tag="ident")
            nc.sync.dma_start(ident[:], d_ident[:])
            kbd = wres.tile([128, 6 * KBD_W], F32R, tag="kbd")
            nc.sync.dma_start(kbd[:], d_kbdz[:])
            vbd = wres.tile([98, 6 * VBD_W], F32R, tag="vbd")
            nc.sync.dma_start(vbd[:], d_vbdz[:])
            bias_sb = wres.tile([1, C], F32R, tag="bias_sb")
            nc.sync.dma_start(bias_sb[:], d_bias[:])
            bias_bc = wres.tile([128, C], F32, tag="bias_bc")
            for half in range(2):
                pb = psB.tile([128, 384], F32, tag="psB")
                nc.tensor.matmul(pb[:], ones1[:],
                                 bias_sb[:, 384 * half:384 * (half + 1)],
                                 start=True, stop=True)
                nc.scalar.copy(bias_bc[:, 384 * half:384 * (half + 1)], pb[:])

            for g in range(N_GROUPS):
                r0 = g * T  # first token row of group

                # ---- A: load x token-major, transpose to xT ----
                x_tm = []
                for (to, tk) in tts:
                    t = p_xtm.tile([128, C], F32, tag="xtm")
                    nc.sync.dma_start(t[:tk, :], d_x[r0 + to:r0 + to + tk, :])
                    x_tm.append(t)
                xT = [p_xT.tile([128, T], F32R, tag=f"xT{ci}", name=f"xT{ci}")
                      for ci in range(6)]
                for tti, (to, tk) in enumerate(tts):
                    for ci in range(6):
                        pt = psB.tile([128, 384], F32, tag="psB")
                        nc.tensor.transpose(
                            pt[:, :tk],
                            x_tm[tti][:tk, 128 * ci:128 * (ci + 1)],
                            ident[:tk, :tk])
                        nc.vector.tensor_copy(xT[ci][:, to:to + tk], pt[:, :tk])

                # ---- B: q,k GEMM; q natural, k scattered block-diag ----
                qk = []
                for j in range(12):
                    pq = psA.tile([128, TP], F32, tag="psA")
                    for ci in range(6):
                        nc.tensor.matmul(
                            pq[:, :T],
                            w_qkv[ci][:, 128 * j:128 * (j + 1)],
                            xT[ci][:, :T],
                            start=(ci == 0), stop=(ci == 5))
                    if j < 6:
                        t = p_qk.tile([128, TP], F32R, tag=f"q{j}", name=f"q{j}")
                        nc.vector.tensor_copy(t[:, :T], pq[:, :T])
                        nc.vector.tensor_copy(t[:, T:T + 2], pq[:, :2])
                        qk.append(t)
                    else:
                        jj = j - 6
                        kv = kbd[:, jj * KBD_W:(jj + 1) * KBD_W].rearrange(
                            "p (b c) -> p b c", c=98)
                        nc.vector.tensor_copy(
                            kv[0:64, :, 0:49],
                            pq[0:64, :T].rearrange("p (b c) -> p b c", c=49))
                        nc.vector.tensor_copy(
                            kv[64:128, :, 49:98],
                            pq[64:128, :T].rearrange("p (b c) -> p b c", c=49))

                # ---- C: v GEMM token-major + block-diag scatter ----
                v4 = vbd.rearrange("p (j b c) -> p j b c", b=G, c=128)
                for tti, (to, tk) in enumerate(tts):
                    scr = p_vscr.tile([128, C], F32R, tag="vscr")
                    for half in range(2):
                        pv = psB.tile([128, 384], F32, tag="psB")
                        for ci in range(6):
                            nc.tensor.matmul(
                                pv[:tk, :],
                                xT[ci][:, to:to + tk],
                                w_qkv[ci][:, 1536 + 384 * half:
                                           1536 + 384 * (half + 1)],
                                start=(ci == 0), stop=(ci == 5))
                        nc.vector.tensor_copy(
                            scr[:tk, 384 * half:384 * (half + 1)], pv[:tk, :])
                    # scatter batch segments of this token tile
                    for b in range(G):
                        lo = max(b * SEQ, to)
                        hi = min((b + 1) * SEQ, to + tk)
                        if lo >= hi:
                            continue
                        sl, sh = lo - b * SEQ, hi - b * SEQ  # rows in block
                        src = scr[lo - to:hi - to, :]
                        # even heads -> rows sl:sh, cols 0:64 of block
                        nc.sync.dma_start(
                            v4[sl:sh, :, b, 0:64],
                            src.rearrange("p (j two c) -> p j two c",
                                          two=2, c=64)[:, :, 0, :])
                        # odd heads -> rows 49+sl:49+sh, cols 64:128
                        nc.sync.dma_start(
                            v4[49 + sl:49 + sh, :, b, 64:128],
                            src.rearrange("p (j two c) -> p j two c",
                                          two=2, c=64)[:, :, 1, :])

                # ---- D: attention, one cell per head pair ----
                unT = [p_unT.tile([128, T], F32R, tag=f"unT{ci}", name=f"unT{ci}")
                       for ci in range(6)]
                rgrp = p_rr.tile([H, T], F32, tag="rgrp")
                for j in range(6):
                    ps = psS.tile([98, 50 * G], F32, tag="psS")
                    for b in range(G):
                        nc.tensor.matmul(
                            ps[:, 50 * b:50 * b + 50],
                            kbd[:, j * KBD_W + 98 * b:j * KBD_W + 98 * b + 98],
                            qk[j][:, 49 * b:49 * b + 50],
                            start=True, stop=True)
                    eT = p_eT.tile([98, 50 * G], F32R, tag="eT")
                    nc.scalar.activation(eT[:], ps[:], EXP, scale=0.125)
                    pr = psB.tile([2, 50 * G], F32, tag="psB")
                    nc.tensor.matmul(pr[:], onesbd[:], eT[:],
                                     start=True, stop=True)
                    po = psO.tile([128, 50 * G], F32, tag="psO")
                    for b in range(G):
                        nc.tensor.matmul(
                            po[:, 50 * b:50 * b + 50],
                            vbd[:, j * VBD_W + 128 * b:j * VBD_W + 128 * (b + 1)],
                            eT[:, 50 * b:50 * b + 50],
                            start=True, stop=True)
                    nc.vector.tensor_copy(
                        unT[j][:, :].rearrange("p (b c) -> p b c", c=49),
                        po[:, :].rearrange("p (b c) -> p b c", c=50)[:, :, 0:49])
                    r2 = p_bc.tile([2, T], F32, tag="r2")
                    nc.scalar.copy(
                        r2.rearrange("p (b c) -> p b c", c=49),
                        pr.rearrange("p (b c) -> p b c", c=50)[:, :, 0:49])
                    nc.sync.dma_start(rgrp[2 * j:2 * j + 2, :], r2[:])
                # one batched reciprocal for all 12 heads of the group
                rr = p_rr.tile([H, T], F32R, tag="rr")
                nc.vector.reciprocal(rr[:], rgrp[:])
                for j in range(6):
                    pbc = psA.tile([128, TP], F32, tag="psA")
                    nc.tensor.matmul(pbc[:, :T], sel2[:, 128 * j:128 * (j + 1)],
                                     rr[:], start=True, stop=True)
                    bc = p_bc.tile([128, T], F32, tag="bc")
                    nc.scalar.copy(bc[:], pbc[:, :T])
                    nc.vector.tensor_mul(out=unT[j][:, :], in0=unT[j][:, :],
                                         in1=bc[:])

                # ---- E: proj GEMM + bias -> out ----
                for tti, (to, tk) in enumerate(tts):
                    osb = p_osb.tile([128, C], F32, tag="osb")
                    for half in range(2):
                        pp = psB.tile([128, 384], F32, tag="psB")
                        for ci in range(6):
                            nc.tensor.matmul(
                                pp[:tk, :],
                                unT[ci][:, to:to + tk],
                                w_proj[ci][:, 384 * half:384 * (half + 1)],
                                start=(ci == 0), stop=(ci == 5))
                        nc.vector.tensor_add(
                            out=osb[:tk, 384 * half:384 * (half + 1)],
                            in0=pp[:tk, :],
                            in1=bias_bc[:tk, 384 * half:384 * (half + 1)])
                    nc.sync.dma_start(d_out[r0 + to:r0 + to + tk, :],
                                      osb[:tk, :])

    nc.compile()
    return nc


def kernel(x, W_qkv, W_proj, b_proj):
    from concourse.bass_utils import run_bass_kernel_spmd

    if "nc" not in _CACHE:
        _CACHE["nc"] = _build()
    nc = _CACHE["nc"]

    x = np.ascontiguousarray(np.asarray(x, dtype=np.float32))
    B, N, Cc = x.shape
    consts = _consts()
    W_qkv = np.ascontiguousarray(np.asarray(W_qkv, dtype=np.float32))
    W_proj = np.ascontiguousarray(np.asarray(W_proj, dtype=np.float32))
    bias = np.ascontiguousarray(
        np.asarray(b_proj, dtype=np.float32).reshape(1, Cc))

    in_maps = []
    for i in range(NUM_CORES):
        m = {"x": np.ascontiguousarray(
                x[i * B_CORE:(i + 1) * B_CORE].reshape(TOK, Cc)),
             "wqkv": W_qkv, "wproj": W_proj, "bias": bias}
        m.update(consts)
        in_maps.append(m)
    res = run_bass_kernel_spmd(nc, in_maps, list(range(NUM_CORES)))
    out = np.empty((B, N, Cc), dtype=np.float32)
    for i in range(NUM_CORES):
        out[i * B_CORE:(i + 1) * B_CORE] = res.results[i]["out"].reshape(
            B_CORE, N, Cc)
    return out
